# revision 28
# baseline (speedup 1.0000x reference)
"""BigBird attention Trainium2 kernel (Bass/Tile), 8-core SPMD.

Sharding: core c -> (batch b = c//4, sequence quarter t = c%4).
Each core computes ALL 16 heads for its 2048 "own" local tokens, plus a
1-block (128 token) halo on each side (recomputed locally, circular) and
the 16 global tokens.  Outputs are disjoint rows of y, so the host gather
is pure concatenation.  The only cross-core communication is a 66 KB
AllReduce of the global-query attention partial sums (numerator+denominator).

v2: bf16 end-to-end (host casts x/weights), fused single-pass q/k/v
projection, software-pipelined attention (PV lagged 2 iters, gx lagged 1),
late-priority tail so the AllReduce wait cannot block independent work.

Device x column layout per core (2320 cols): [own 2048 | hl 128 | hr 128 | g 16].
"""

import numpy as np
import ml_dtypes

# ---------------- problem constants (hardcoded per contract) ----------------
D_MODEL = 1024
H = 16
DK = 64
DV = 64
BLOCK = 128
G = 16
B = 2
T = G + 8192          # 8208
NBLK = 64             # local blocks per batch
QB = 16               # own q blocks per core
T_OWN = QB * BLOCK    # 2048
XC = T_OWN + 2 * BLOCK + G  # 2320 device x cols: [own | hl | hr | g]
N_CORES = 8
P = 128
KC = D_MODEL // P     # 8 contraction chunks
MC = (H * DK) // P    # 8 row chunks of qT/kT (2 heads per chunk)
SCALE = 1.0 / 8.0     # 1/sqrt(64)

# column offsets in the device-x layout
OWN0 = 0
HL0 = T_OWN            # 2048
HR0 = T_OWN + BLOCK    # 2176
G0 = T_OWN + 2 * BLOCK # 2304 (globals in kT / x layout)
QXC = T_OWN + G        # 2064 qT cols: [own | g]
QG0 = T_OWN            # globals offset within qT


def _kcols(r):
    """Columns of k-block with relative index r in [-1, 16]."""
    if r == -1:
        return HL0
    if r == 16:
        return HR0
    return r * BLOCK


def _vblk(r):
    """v_sb block index for relative k-block r."""
    if r == -1:
        return 16
    if r == 16:
        return 17
    return r


def build_program():
    import concourse.bacc as bacc
    import concourse.tile as tile
    import concourse.mybir as mybir
    from concourse.masks import make_identity
    from contextlib import ExitStack

    dt = mybir.dt
    F32 = dt.float32
    BF = dt.bfloat16
    Exp = mybir.ActivationFunctionType.Exp
    Copy = mybir.ActivationFunctionType.Copy

    nc = bacc.Bacc("TRN2", target_bir_lowering=False, debug=False,
                   num_devices=N_CORES)

    # ---------------- external I/O (bf16 inputs, f32 outputs) ----------------
    xT_d = nc.dram_tensor("xin", [P, KC, XC], BF, kind="ExternalInput").ap()
    wqT_d = nc.dram_tensor("wqT", [P, KC, H * DK], BF, kind="ExternalInput").ap()
    wkT_d = nc.dram_tensor("wkT", [P, KC, H * DK], BF, kind="ExternalInput").ap()
    wvT_d = nc.dram_tensor("wvT", [P, KC, H * DV], BF, kind="ExternalInput").ap()
    woT_d = nc.dram_tensor("woT", [P, KC, D_MODEL], BF, kind="ExternalInput").ap()
    bo_d = nc.dram_tensor("bo", [1, D_MODEL], BF, kind="ExternalInput").ap()
    y_own_d = nc.dram_tensor("y_own", [T_OWN, D_MODEL], BF,
                             kind="ExternalOutput").ap()
    y_g_d = nc.dram_tensor("y_g", [G, D_MODEL], BF, kind="ExternalOutput").ap()

    with tile.TileContext(nc) as tc, ExitStack() as top:
        # ------------- persistent SBUF -------------
        pool_qT = top.enter_context(tc.tile_pool(name="qT", bufs=1))
        pool_kT = top.enter_context(tc.tile_pool(name="kT", bufs=1))
        pool_v = top.enter_context(tc.tile_pool(name="v", bufs=1))
        pool_misc = top.enter_context(tc.tile_pool(name="misc", bufs=1))
        qT_sb = pool_qT.tile([P, MC, QXC], BF)       # rows (h,d) chunked, cols t
        kT_sb = pool_kT.tile([P, MC, XC], BF)
        v_sb = pool_v.tile([P, 18, H, 65], BF)       # [row%128, kblk, h, d(+1)]
        vg_sb = pool_misc.tile([P, H, 65], BF)       # global v, replicated at
        gx_sb = pool_misc.tile([G, H, 65], F32)      # partition bands 0/32/64/96
        nc.gpsimd.memset(v_sb[:, :, :, 64:65], 1.0)
        nc.gpsimd.memset(vg_sb[:, :, 64:65], 1.0)

        # DRAM bounce buffers for the gx AllReduce
        pool_dram = top.enter_context(tc.tile_pool(name="dram", bufs=1, space="DRAM"))
        gx_part_d = pool_dram.tile([G, H, 65], F32)
        gx_full_d = pool_dram.tile([G, H, 65], F32)

        # ------- phase 1: fused q,k,v projections (single pass over x) -------
        NW = 512
        segs = []
        o = 0
        while o < XC:
            w = min(NW, XC - o)
            segs.append((o, w))
            o += w

        with ExitStack() as s1:
            pool_w1 = s1.enter_context(tc.tile_pool(name="w1", bufs=1))
            pool_x1 = s1.enter_context(tc.tile_pool(name="x1", bufs=3))
            pool_psk = s1.enter_context(tc.tile_pool(name="psk", bufs=2, space="PSUM"))
            pool_psq = s1.enter_context(tc.tile_pool(name="psq", bufs=2, space="PSUM"))
            pool_psv = s1.enter_context(tc.tile_pool(name="psv", bufs=2, space="PSUM"))
            wk_sb = pool_w1.tile([P, KC, H * DK], BF, name="wk")
            wq_sb = pool_w1.tile([P, KC, H * DK], BF, name="wq")
            wv_sb = pool_w1.tile([P, KC, H * DV], BF, name="wv")
            # weight DMAs spread over the DMA-capable queues, v first (the
            # segment loop consumes wv first so the PE can start earliest)
            nc.scalar.dma_start(out=wv_sb[:, :, 0:512], in_=wvT_d[:, :, 0:512])
            nc.scalar.dma_start(out=wv_sb[:, :, 512:1024],
                                in_=wvT_d[:, :, 512:1024])
            nc.gpsimd.dma_start(out=wk_sb[:, :, 0:512], in_=wkT_d[:, :, 0:512])
            nc.gpsimd.dma_start(out=wk_sb[:, :, 512:1024],
                                in_=wkT_d[:, :, 512:1024])
            nc.scalar.dma_start(out=wq_sb[:], in_=wqT_d[:])
            for src0, nw in segs:
                a, b = src0, src0 + nw
                xt = pool_x1.tile([P, KC, NW], BF, tag="xt", name="xt")
                nc.sync.dma_start(out=xt[:, :, :nw],
                                  in_=xT_d[:, :, src0:src0 + nw])
                # ---- v projection (x-block stationary) ----
                nblk_seg = (nw + P - 1) // P
                for blk in range(nblk_seg):
                    rows = min(P, nw - blk * P)
                    m = (src0 + blk * P) // P     # 0..17 local, 18 = globals
                    for nv in range(2):
                        ps = pool_psv.tile([P, 512], F32, tag="psv", name="psv")
                        for kc in range(KC):
                            nc.tensor.matmul(
                                ps[:rows, :],
                                lhsT=xt[:, kc, blk * P:blk * P + rows],
                                rhs=wv_sb[:, kc, nv * 512:(nv + 1) * 512],
                                start=(kc == 0), stop=(kc == KC - 1))
                        srcv = ps[:rows, :].rearrange("p (h d) -> p h d", h=8)
                        if m < 18:
                            dstv = v_sb[:rows, m, nv * 8:(nv + 1) * 8, 0:64]
                        else:
                            dstv = vg_sb[:rows, nv * 8:(nv + 1) * 8, 0:64]
                        nc.vector.tensor_copy(dstv, srcv)
                # ---- k projection ----
                for mc in range(MC):
                    ps = pool_psk.tile([P, NW], F32, tag="psk", name="psk")
                    for kc in range(KC):
                        nc.tensor.matmul(
                            ps[:, :nw],
                            lhsT=wk_sb[:, kc, mc * P:(mc + 1) * P],
                            rhs=xt[:, kc, :nw],
                            start=(kc == 0), stop=(kc == KC - 1))
                    nc.vector.tensor_copy(kT_sb[:, mc, src0:src0 + nw],
                                          ps[:, :nw])
                # ---- q projection (own cols + global cols) ----
                qparts = []
                if a < T_OWN:
                    qparts.append((0, min(b, T_OWN) - a, a))
                if b > G0:
                    s_ = max(a, G0)
                    qparts.append((s_ - a, b - s_, QG0 + s_ - G0))
                for xoff, pw, dst0 in qparts:
                    for mc in range(MC):
                        ps = pool_psq.tile([P, NW], F32, tag="psq", name="psq")
                        for kc in range(KC):
                            nc.tensor.matmul(
                                ps[:, :pw],
                                lhsT=wq_sb[:, kc, mc * P:(mc + 1) * P],
                                rhs=xt[:, kc, xoff:xoff + pw],
                                start=(kc == 0), stop=(kc == KC - 1))
                        nc.scalar.activation(qT_sb[:, mc, dst0:dst0 + pw],
                                             ps[:, :pw], Copy)

        # ---------------- phases 2+3 ----------------
        s23 = top.enter_context(ExitStack())
        pool_outx = s23.enter_context(tc.tile_pool(name="outx", bufs=1))
        out_x = pool_outx.tile([P, QB, H * DV], BF)
        pool_wo = s23.enter_context(tc.tile_pool(name="wo", bufs=1))
        wo_sb = pool_wo.tile([P, KC, D_MODEL], BF)
        bo_sb = pool_wo.tile([1, D_MODEL], BF)
        nc.gpsimd.dma_start(out=wo_sb[:], in_=woT_d[:])
        nc.gpsimd.dma_start(out=bo_sb[:], in_=bo_d[:])
        # ---------------- phase 2: attention ----------------
        with ExitStack() as s3:
            pool_probs = s3.enter_context(tc.tile_pool(name="probs", bufs=7))
            pool_pxg = s3.enter_context(tc.tile_pool(name="pxg", bufs=3))
            pool_ps_s = s3.enter_context(tc.tile_pool(name="ps_s", bufs=3, space="PSUM"))
            pool_ps_o = s3.enter_context(tc.tile_pool(name="ps_o", bufs=3, space="PSUM"))
            pool_ps_gx = s3.enter_context(tc.tile_pool(name="ps_gx", bufs=1, space="PSUM"))
            pool_nrm = s3.enter_context(tc.tile_pool(name="nrm", bufs=3))

            pxg4s = {}

            def qkh(sb, h, c0, c1):
                hb = 64 * (h % 2)
                return sb[hb:hb + 64, h // 2, c0:c1]

            def xg_block(g4):
                """xg scores+exp for the 4 heads 4*g4..4*g4+3, packed into
                row bands 0/32/64/96 of one PSUM tile so one Exp covers all
                four heads."""
                heads4 = range(4 * g4, 4 * g4 + 4)
                pxg0 = [pool_pxg.tile([G, 2, 4, 512], BF, tag="pxg0",
                                      name="pxg0") for _ in range(2)]
                for nq in range(4):
                    psx = pool_ps_s.tile([P, 512], F32, tag="ps_s",
                                         name="psx")
                    pxg4 = pool_pxg.tile([P, 512], BF, tag="pxg4",
                                         name="pxg4")
                    for h in heads4:
                        s = h % 4
                        nc.tensor.matmul(psx[32 * s:32 * s + G, :],
                                         lhsT=qkh(kT_sb, h, G0, G0 + G),
                                         rhs=qkh(qT_sb, h, nq * 512,
                                                 (nq + 1) * 512),
                                         start=True, stop=True,
                                         tile_position=(64 * (h % 2), 32 * s))
                    nc.scalar.activation(pxg4[:, :], psx[:, :],
                                         Exp, scale=SCALE)
                    # move the 4 packed bands back to partition base 0 so PV
                    # stationaries load from base 0 (fast path)
                    p4v = pxg4[:].rearrange("(s g) c -> s g c", s=4, g=32)
                    for s in range(4):
                        nc.sync.dma_start(
                            out=pxg0[s % 2][:, s // 2, nq, :],
                            in_=p4v[s, 0:G, :])
                pxg4s[g4] = pxg0

            def pxg_ap(h, i):
                """[16, 128] probs slice of global-k scores for (head, qblk)."""
                s = h % 4
                return pxg4s[h // 4][s % 2][:, s // 2, i // 4,
                                            (i % 4) * BLOCK:
                                            (i % 4) * BLOCK + BLOCK]

            for hp2 in range(H // 2):    # head pairs (2*hp2, 2*hp2+1)
                heads = (2 * hp2, 2 * hp2 + 1)
                if hp2 % 2 == 0:
                    xg_block(hp2 // 2)

                # gx accumulators: one PSUM bank per head (interleaved
                # accumulation chains must not share a bank: each chain's
                # first matmul clears has_written for the whole bank)
                ps_gx_t = {sub: pool_ps_gx.tile([G, 65], F32,
                                                tag=f"psgx{sub}",
                                                name="ps_gx")
                           for sub in range(2)}
                probs = {}

                def do_pv(i, ps_po):
                    """PV + normalization for q-block i, both heads in one
                    PSUM tile: [128, (sub, a, 65)] a=0 local, a=1 global."""
                    po3 = ps_po[:].rearrange("p (s a b) -> p (s a) b",
                                             s=2, a=2, b=65)
                    for sub, h in enumerate(heads):
                        for dj, j in enumerate((i - 1, i, i + 1)):
                            pj, psub, jlo, _ = probs[(h, j)]
                            c0 = (i - jlo) * BLOCK
                            nc.tensor.matmul(po3[:, 2 * sub, :],
                                             lhsT=pj[:, psub, c0:c0 + BLOCK],
                                             rhs=v_sb[:, _vblk(j), h, 0:65],
                                             start=(dj == 0 and sub == 0),
                                             stop=(dj == 2),
                                             skip_group_check=True)
                        nc.tensor.matmul(po3[:, 2 * sub + 1, :],
                                         lhsT=pxg_ap(h, i),
                                         rhs=vg_sb[0:G, h, 0:65],
                                         start=(sub == 0), stop=True,
                                         skip_group_check=True)
                    rec = pool_nrm.tile([P, 4], F32, tag="rec", name="rec")
                    dns = ps_po[:].rearrange("p (c b) -> p c b", b=65)[:, :, 64]
                    nc.vector.reciprocal(rec[:, :], dns)
                    for sub, h in enumerate(heads):
                        tG = pool_nrm.tile([P, DV], BF, tag=f"tG{sub}",
                                           name="tG")
                        if sub == 0:
                            nc.scalar.activation(tG[:],
                                                 po3[:, 2 * sub + 1, 0:64],
                                                 Copy, scale=rec[:, 2 * sub + 1:
                                                                 2 * sub + 2])
                        else:
                            nc.vector.tensor_scalar_mul(
                                tG[:], po3[:, 2 * sub + 1, 0:64],
                                rec[:, 2 * sub + 1:2 * sub + 2])
                        nc.vector.scalar_tensor_tensor(
                            out_x[:, i, h * DV:(h + 1) * DV],
                            po3[:, 2 * sub, 0:64], rec[:, 2 * sub:2 * sub + 1],
                            tG[:],
                            op0=mybir.AluOpType.mult, op1=mybir.AluOpType.add)

                for r_ in range(-1, 18):
                    if r_ <= 16:
                        # scores for k-block r_ for BOTH heads of the pair:
                        # adjacent MMs at partition bases 0/64 run concurrently
                        # in different PE row groups.
                        ilo, ihi = max(r_ - 1, 0), min(r_ + 1, QB - 1)
                        nloc = (ihi - ilo + 1) * BLOCK
                        own = 0 <= r_ <= 15
                        ntot = nloc + (G if own else 0)
                        kc0 = _kcols(r_)
                        pt = pool_probs.tile([P, 2, 512], BF, tag="probs",
                                             name="pt")
                        pss = {}
                        merged = own and ihi == QB - 1
                        for sub, h in enumerate(heads):
                            ps_s = pool_ps_s.tile([P, 512], F32, tag="ps_s",
                                                  name="ps_s")
                            pss[sub] = ps_s
                            nc.tensor.matmul(ps_s[:, 0:ntot if merged
                                                  else nloc],
                                             lhsT=qkh(kT_sb, h, kc0,
                                                      kc0 + BLOCK),
                                             rhs=qkh(qT_sb, h, ilo * BLOCK,
                                                     (ihi + 1) * BLOCK +
                                                     (G if merged else 0)),
                                             start=True, stop=True)
                            if own and not merged:
                                nc.tensor.matmul(ps_s[:, nloc:ntot],
                                                 lhsT=qkh(kT_sb, h, kc0,
                                                          kc0 + BLOCK),
                                                 rhs=qkh(qT_sb, h, QG0,
                                                         QG0 + G),
                                                 start=False, stop=True,
                                                 skip_group_check=True)
                            probs[(h, r_)] = (pt, sub, ilo, nloc)
                        for sub, h in enumerate(heads):
                            nc.scalar.activation(pt[:, sub, 0:ntot],
                                                 pss[sub][:, 0:ntot],
                                                 Exp, scale=SCALE)
                    # gx accumulation (global rows), lagged one iter; flipped
                    # so the stationary is the tiny [128,16] probs slice.
                    if 1 <= r_ <= 16:
                        j = r_ - 1
                        for sub, h in enumerate(heads):
                            pj, psub, jlo, jnloc = probs[(h, j)]
                            nc.tensor.matmul(
                                ps_gx_t[sub][:, :],
                                lhsT=pj[:, psub, jnloc:jnloc + G],
                                rhs=v_sb[:, j, h, 0:65],
                                start=(j == 0), stop=(j == 15))
                    # PV lagged two iters: probs for i-1,i,i+1 all ready
                    i = r_ - 2
                    if 0 <= i <= QB - 1:
                        ps_po = pool_ps_o.tile([P, 260], F32, tag="ps_o",
                                               name="ps_o")
                        do_pv(i, ps_po)
                    for key in list(probs):
                        if key[1] < r_ - 3:
                            probs.pop(key)
                # stash gx partials for both heads
                for sub, h in enumerate(heads):
                    nc.vector.tensor_copy(gx_sb[:, h, :], ps_gx_t[sub][:, :])

            nc.sync.dma_start(out=gx_part_d[:], in_=gx_sb[:])
            nc.gpsimd.collective_compute(
                "AllReduce", mybir.AluOpType.add,
                replica_groups=[[0, 1, 2, 3], [4, 5, 6, 7]],
                ins=[gx_part_d.opt()], outs=[gx_full_d.opt()])

        # ---------------- phase 3: output projection ----------------
        with ExitStack() as s4:
            pool_wo2 = s4.enter_context(tc.tile_pool(name="wo2", bufs=1))
            pool_ot = s4.enter_context(tc.tile_pool(name="ot", bufs=10))
            pool_pst = s4.enter_context(tc.tile_pool(name="pst", bufs=3, space="PSUM"))
            pool_psy = s4.enter_context(tc.tile_pool(name="psy", bufs=2, space="PSUM"))
            pool_ysb = s4.enter_context(tc.tile_pool(name="ysb", bufs=3))
            pool_gxf = s4.enter_context(tc.tile_pool(name="gxf", bufs=1))
            ones1 = pool_wo2.tile([1, P], BF)
            bias_sb = pool_wo2.tile([P, D_MODEL], F32)
            ident = pool_wo2.tile([P, P], BF)
            nc.vector.memset(ones1[:], 1.0)
            make_identity(nc, ident[:])
            for nv in range(2):
                psb0 = pool_psy.tile([P, 512], F32, tag="psy")
                nc.tensor.matmul(psb0[:], lhsT=ones1[:],
                                 rhs=bo_sb[:, nv * 512:(nv + 1) * 512],
                                 start=True, stop=True)
                nc.scalar.activation(bias_sb[:, nv * 512:(nv + 1) * 512],
                                     psb0[:], Copy)

            for m in range(QB):
                ots = []
                for kc in range(KC):
                    pst = pool_pst.tile([P, P], BF, tag="pst")
                    nc.tensor.transpose(pst[:],
                                        out_x[:, m, kc * P:(kc + 1) * P],
                                        ident[:])
                    ot = pool_ot.tile([P, P], BF, tag="ot")
                    if kc % 2 == 0:
                        nc.scalar.activation(ot[:], pst[:], Copy)
                    else:
                        nc.vector.tensor_copy(ot[:], pst[:])
                    ots.append(ot)
                for nv in range(2):
                    psy = pool_psy.tile([P, 512], F32, tag="psy")
                    for kc in range(KC):
                        nc.tensor.matmul(psy[:],
                                         lhsT=ots[kc][:],
                                         rhs=wo_sb[:, kc, nv * 512:(nv + 1) * 512],
                                         start=(kc == 0), stop=(kc == KC - 1))
                    ysb = pool_ysb.tile([P, 512], BF, tag="ysb")
                    nc.vector.tensor_add(ysb[:], psy[:],
                                         bias_sb[:, nv * 512:(nv + 1) * 512])
                    nc.sync.dma_start(
                        out=y_own_d[m * P:(m + 1) * P, nv * 512:(nv + 1) * 512],
                        in_=ysb[:])

            # ----- global rows: normalize gx and project (after AllReduce).
            # Late priority: nothing here may steal queue slots ahead of the
            # independent y_own work above (the AllReduce wait would then
            # head-of-line-block it).
            with tc.high_priority(offset=-(1 << 22)):
                gxf = pool_gxf.tile([G, H, 65], F32)
                rden = pool_gxf.tile([G, H], F32)
                norm4 = pool_gxf.tile([G, H, DV], BF)    # [g, h, d]
                norm_sbT = pool_gxf.tile([P, KC, G], BF)  # [(h d), kc, g]
                nc.sync.dma_start(out=gxf[:], in_=gx_full_d[:])
                nc.vector.reciprocal(rden[:], gxf[:, :, 64])
                for h in range(H):
                    nc.vector.tensor_scalar_mul(norm4[:, h, :],
                                                gxf[:, h, 0:64],
                                                rden[:, h:h + 1])
                n4f = norm4[:].rearrange("g h d -> g (h d)")
                for kc in range(KC):
                    pst = pool_pst.tile([P, G], BF, tag="pst")
                    nc.tensor.transpose(pst[:, :],
                                        n4f[:, kc * P:(kc + 1) * P],
                                        ident[0:G, 0:G])
                    nc.scalar.activation(norm_sbT[:, kc, :], pst[:, :], Copy)
                for nv in range(2):
                    psy = pool_psy.tile([G, 512], F32, tag="psy")
                    for kc in range(KC):
                        nc.tensor.matmul(psy[:],
                                         lhsT=norm_sbT[:, kc, :],
                                         rhs=wo_sb[:, kc,
                                                   nv * 512:(nv + 1) * 512],
                                         start=(kc == 0), stop=(kc == KC - 1))
                    ygsb = pool_ysb.tile([G, 512], BF, tag="ygsb")
                    nc.vector.tensor_add(ygsb[:], psy[:],
                                         bias_sb[0:G, nv * 512:(nv + 1) * 512])
                    nc.sync.dma_start(out=y_g_d[:, nv * 512:(nv + 1) * 512],
                                      in_=ygsb[:])

    nc.compile()
    return nc


def shard_inputs(x, Wq, Wk, Wv, Wo, bo):
    """Build the 8 per-core input maps (bf16)."""
    bf16 = ml_dtypes.bfloat16
    x = np.asarray(x, dtype=np.float32).astype(bf16)
    def wprep(W, n):
        return np.ascontiguousarray(
            np.asarray(W, np.float32).astype(bf16).T
            .reshape(KC, P, n).transpose(1, 0, 2))
    wqT = wprep(Wq, H * DK)
    wkT = wprep(Wk, H * DK)
    wvT = wprep(Wv, H * DV)
    woT = wprep(Wo, D_MODEL)
    bo2 = np.asarray(bo, np.float32).astype(bf16).reshape(1, D_MODEL)
    in_maps = []
    for c in range(N_CORES):
        b, t = c // 4, c % 4
        xg = x[b, :G]                       # [16, 1024]
        xl = x[b, G:]                       # [8192, 1024]
        own = xl[t * T_OWN:(t + 1) * T_OWN]
        hl = xl[((16 * t - 1) % NBLK) * BLOCK:][:BLOCK]
        hr = xl[((16 * t + 16) % NBLK) * BLOCK:][:BLOCK]
        xc = np.concatenate([own, hl, hr, xg], axis=0)          # [2320, 1024]
        xT = np.ascontiguousarray(
            xc.T.reshape(KC, P, XC).transpose(1, 0, 2))         # [128, 8, 2320]
        in_maps.append({"xin": xT, "wqT": wqT, "wkT": wkT, "wvT": wvT,
                        "woT": woT, "bo": bo2})
    return in_maps


_NC_CACHE = {}


def get_program():
    if "nc" not in _NC_CACHE:
        _NC_CACHE["nc"] = build_program()
    return _NC_CACHE["nc"]


def _install_ntff_hook():
    """Provide antenv.axon_hooks (missing in this image) so that
    run_bass_kernel_spmd(trace=True) can capture NTFF profiles."""
    import sys, types
    if "antenv.axon_hooks" in sys.modules:
        return
    try:
        import antenv  # noqa: F401
        from trn_agent_boot.trn_boot import _ntff_profile_via_ctypes
        mod = types.ModuleType("antenv.axon_hooks")
        mod._hook = _ntff_profile_via_ctypes("/opt/axon/libaxon_pjrt.so")
        mod.set_axon_ntff_profile_hook = lambda h: setattr(mod, "_hook", h)
        mod.get_axon_ntff_profile_hook = lambda: mod._hook
        sys.modules["antenv.axon_hooks"] = mod
    except Exception as e:  # profiling is optional
        print(f"ntff hook install failed: {e}")


def run(x, Wq, Wk, Wv, Wo, bo, trace=False):
    from concourse.bass_utils import run_bass_kernel_spmd
    if trace:
        _install_ntff_hook()
    nc = get_program()
    in_maps = shard_inputs(x, Wq, Wk, Wv, Wo, bo)
    res = run_bass_kernel_spmd(nc, in_maps, list(range(N_CORES)), trace=trace)
    y = np.empty((B, T, D_MODEL), dtype=np.float32)
    for c in range(N_CORES):
        b, t = c // 4, c % 4
        if t == 0:
            y[b, :G] = res.results[c]["y_g"]
        y[b, G + t * T_OWN:G + (t + 1) * T_OWN] = res.results[c]["y_own"]
    return y, res


def kernel(x, Wq, Wk, Wv, Wo, bo):
    y, _ = run(x, Wq, Wk, Wv, Wo, bo, trace=False)
    return y
